# revision 32
# baseline (speedup 1.0000x reference)
"""HGNN layer (hypergraph message passing) Trainium2 kernel, 8 NeuronCores.

Sharding: one graph per PAIR of cores (4 graphs x 2 cores), output-split:
within a pair each core owns HALF of every stage's output rows/columns.
Each stage streams its big-matrix shard as the MOVING matmul operand in
2MB chunks over HWDGE while the [*,128] intermediate sits stationary in
bf16 vN/vE tile form. The incidence matrix H is 0/1 so its two layouts
(hcol, htc) stream as fp8_e4m3 (exact, half the bytes); degree matrices
stream bf16. All streamed tensors are row-permuted OWN-HALF-FIRST on the
host so each stage accumulates its own 16 k-tiles while the pair exchange
for the partner half is still in flight. Exchanges are pair AllReduces of
the own-half bf16 tile; the partner half is recovered symmetrically as
(sum - own). The softmax normalizer z rides in the first exchange as a
hi/lo bf16 pair. x@W+b and x@theta are precomputed on the host.
"""

import numpy as np

B, N, E, D = 4, 4096, 4096, 128
HALF = N // 2
NCORES = 8
PAIRS = [[0, 1], [2, 3], [4, 5], [6, 7]]
BN_EPS = 1e-5
F = 512                 # moving free-dim per matmul
NT = N // 128           # 32 k-tiles over a full 4096 dim
HT = HALF // 128        # 16 tiles over a half (own output)
CH8 = 4                 # k-tiles per fp8 chunk (1MB)
CH16 = 2                # k-tiles per bf16 chunk (1MB)
ZW = 8                  # rider columns appended to exchange 1

_CACHE = {}


def _build():
    import concourse.bacc as bacc
    import concourse.mybir as mybir
    import concourse.tile as tile
    from concourse.masks import make_identity
    from contextlib import ExitStack

    fp32 = mybir.dt.float32
    bf16 = mybir.dt.bfloat16
    fp8 = mybir.dt.float8e4
    Act = mybir.ActivationFunctionType
    Alu = mybir.AluOpType

    nc = bacc.Bacc("TRN2", target_bir_lowering=False, debug=False,
                   num_devices=NCORES)

    # ---- per-core DRAM inputs (shards; see kernel() for host layout) ----
    xwv_d = nc.dram_tensor("xwv", [128, NT * 128], bf16, kind="ExternalInput")
    sc_d = nc.dram_tensor("sc", [128, HT], fp32, kind="ExternalInput")
    hcol_d = nc.dram_tensor("hcol", [N, HALF], fp8, kind="ExternalInput")
    htc_d = nc.dram_tensor("htc", [E, HALF], fp8, kind="ExternalInput")
    dvT_d = nc.dram_tensor("dvT", [N, HALF], bf16, kind="ExternalInput")
    deT_d = nc.dram_tensor("deT", [E, HALF], bf16, kind="ExternalInput")
    mask_d = nc.dram_tensor("mask", [128, HT], fp32, kind="ExternalInput")
    eps_d = nc.dram_tensor("eps", [D, 1], fp32, kind="ExternalInput")
    bng_d = nc.dram_tensor("bng", [D, 1], fp32, kind="ExternalInput")
    bnb_d = nc.dram_tensor("bnb", [D, 1], fp32, kind="ExternalInput")
    bnm_d = nc.dram_tensor("bnm", [D, 1], fp32, kind="ExternalInput")
    bnv_d = nc.dram_tensor("bnv", [D, 1], fp32, kind="ExternalInput")
    y_d = nc.dram_tensor("y", [D, HALF], fp32, kind="ExternalOutput")

    with tile.TileContext(nc) as tc, ExitStack() as ctx:
        const = ctx.enter_context(tc.tile_pool(name="const", bufs=1))
        stream = ctx.enter_context(tc.tile_pool(name="stream", bufs=10))
        med = ctx.enter_context(tc.tile_pool(name="med", bufs=1))
        small = ctx.enter_context(tc.tile_pool(name="small", bufs=1))
        ps = ctx.enter_context(tc.tile_pool(name="ps", bufs=8, space="PSUM"))
        dram = ctx.enter_context(tc.tile_pool(name="dram", bufs=1, space="DRAM"))

        ident = const.tile([128, 128], fp32)
        make_identity(nc, ident)
        ones2 = const.tile([2, 1], fp32)
        nc.vector.memset(ones2[:], 1.0)
        onesc = const.tile([128, 1], fp32)
        nc.vector.memset(onesc[:], 1.0)
        ones_row = const.tile([1, 128], fp32)
        nc.vector.memset(ones_row[:], 1.0)

        def load_param(dt_):
            t = const.tile([D, 1], fp32, tag=dt_.name + "_p")
            nc.sync.dma_start(out=t[:], in_=dt_.ap())
            return t

        eps_t = load_param(eps_d)
        bng_t = load_param(bng_d)
        bnb_t = load_param(bnb_d)
        bnm_t = load_param(bnm_d)
        bnv_t = load_param(bnv_d)
        maskc = const.tile([128, HT], fp32)
        nc.sync.dma_start(out=maskc[:], in_=mask_d.ap())

        # bn scale s = gamma * rsqrt(var + eps_bn); shift t = beta - mean*s
        s_bn = small.tile([D, 1], fp32, tag="s_bn")
        nc.vector.tensor_scalar_add(s_bn[:], bnv_t[:], BN_EPS)
        nc.scalar.activation(s_bn[:], s_bn[:], Act.Sqrt)
        nc.vector.reciprocal(s_bn[:], s_bn[:])
        nc.vector.tensor_mul(s_bn[:], s_bn[:], bng_t[:])
        t_bn = small.tile([D, 1], fp32, tag="t_bn")
        nc.vector.tensor_mul(t_bn[:], bnm_t[:], s_bn[:])
        nc.vector.tensor_tensor(t_bn[:], bnb_t[:], t_bn[:], op=Alu.subtract)

        def transpose_cols(src, j, out_ap, scale=None):
            """PE-transpose src[:, 128j:128(j+1)] -> out_ap (optionally
            scaled per-partition by `scale` [128,1]) via psum."""
            pt = ps.tile([128, 128], fp32, tag="ps", name="pt")
            nc.tensor.transpose(pt[:], src[:, j * 128:(j + 1) * 128], ident[:])
            if scale is None:
                nc.vector.tensor_copy(out_ap, pt[:])
            else:
                nc.vector.tensor_scalar_mul(out_ap, pt[:], scale)

        # ------- stage 1 (host-precomputed): load x_wv vN tiles + xth hi/lo
        x_wv = med.tile([D, N], bf16, tag="x_wv")
        nc.sync.dma_start(out=x_wv[:], in_=xwv_d.ap())
        sc_t = const.tile([128, HT], fp32)
        nc.sync.dma_start(out=sc_t[:], in_=sc_d.ap())

        # ------- generic streamed stage: acc[d, own] += stat(j).T @ M[j] ---
        def stream_stage(dram_t, dt, ch, stat_fn, name, extra=None):
            accs = [ps.tile([128, F], fp32, tag="ps", name=f"{name}_a{i}")
                    for i in range(HALF // F)]
            for c in range(NT // ch):
                chk = stream.tile([128, ch * HALF], dt, tag="stream",
                                  name="chk")
                nc.sync.dma_start(
                    out=chk[:].rearrange("p (g n) -> p g n", g=ch),
                    in_=dram_t.ap()[c * ch * 128:(c + 1) * ch * 128, :]
                        .rearrange("(g p) n -> p g n", g=ch))
                for jj in range(ch):
                    j = c * ch + jj
                    st, sp = (j == 0), (j == NT - 1)
                    for blk in range(HALF // F):
                        sl = slice(jj * HALF + blk * F,
                                   jj * HALF + (blk + 1) * F)
                        nc.tensor.matmul(accs[blk][:], stat_fn(j), chk[:, sl],
                                         start=st, stop=sp)
                    if extra is not None:
                        extra(j, chk, jj)
            return accs

        # ------- exchange: pair-AllReduce of own-half bf16 tile ------------
        # Returns (own_ap_fn, partner tile); partner = sum - own.
        def exchange(ex, width, name, ab):
            ci = dram.tile([128, width], bf16, tag=f"{name}_i",
                           name=f"{name}_i")
            co = dram.tile([128, width], bf16, tag=f"{name}_o",
                           name=f"{name}_o")
            nc.sync.dma_start(out=ci[:], in_=ex[:])
            nc.gpsimd.collective_compute(
                "AllReduce", Alu.add, replica_groups=PAIRS,
                ins=[ci.opt()], outs=[co.opt()])
            summ = med.tile([128, width], bf16, tag=f"sum{ab}",
                            name=f"{name}_s")
            part = med.tile([128, width], bf16, tag=f"pr{ab}",
                            name=f"{name}_pr")
            # readback + subtract in halves: the first partner tiles
            # unblock without waiting for the full 0.5MB readback
            hw = width // 2
            for h2 in range(2):
                sl = slice(h2 * hw, width if h2 else hw)
                nc.sync.dma_start(out=summ[:, sl], in_=co[:, sl])
                nc.vector.tensor_tensor(part[:, sl], summ[:, sl],
                                        ex[:, sl], op=Alu.subtract)
            return part

        def split_stat(ex, part):
            def fn(j):
                src = ex if j < HT else part
                t = j % HT
                return src[:, t * 128:(t + 1) * 128]
            return fn

        # ------- stage 2: hxT[d, e'] = (Ht@x_w).T own e-half --------------
        hx_ps = stream_stage(
            hcol_d, fp8, CH8,
            lambda j: x_wv[:, j * 128:(j + 1) * 128], "hx")
        hxT = med.tile([D, HALF], fp32, tag="hxT")
        for blk in range(HALF // F):
            sl = slice(blk * F, (blk + 1) * F)
            nc.vector.tensor_copy(hxT[:, sl], hx_ps[blk][:])

        # softmax: scores precomputed on host in column form [128, HT]
        attnv = med.tile([128, HT], fp32, tag="attnv")
        nc.scalar.activation(attnv[:], sc_t[:], Act.Exp)
        nc.vector.tensor_mul(attnv[:], attnv[:], maskc[:])
        zps = ps.tile([1, HT], fp32, tag="ps", name="zps")
        nc.tensor.matmul(zps[:], onesc[:], attnv[:], start=True, stop=True)
        zrow = small.tile([1, HT], fp32, tag="zrow")
        nc.vector.tensor_copy(zrow[:], zps[:])
        z_t = small.tile([1, 1], fp32, tag="z_t")
        nc.vector.reduce_sum(z_t[:], zrow[:], axis=mybir.AxisListType.X)

        # eps-scaled hxT for stage 10
        ehxT = med.tile([D, HALF], fp32, tag="ehxT")
        nc.vector.tensor_scalar_mul(ehxT[:], hxT[:], eps_t[:])

        # h1a own vE tiles (bf16), z hi/lo riding in cols HALF..HALF+1
        exa = med.tile([128, HALF + ZW], bf16, tag="exA")
        for t in range(HT):
            transpose_cols(hxT[:], t, exa[:, t * 128:(t + 1) * 128],
                           scale=attnv[:, t:t + 1])
        nc.vector.memset(exa[:, HALF:], 0.0)
        nc.vector.tensor_copy(exa[0:1, HALF:HALF + 1], z_t[:])
        zt2 = small.tile([1, 1], fp32, tag="zt2")
        nc.vector.tensor_copy(zt2[:], exa[0:1, HALF:HALF + 1])
        nc.vector.tensor_tensor(zt2[:], z_t[:], zt2[:], op=Alu.subtract)
        nc.vector.tensor_copy(exa[0:1, HALF + 1:HALF + 2], zt2[:])

        h1ap = exchange(exa, HALF + ZW, "ex1", "A")

        # rz = 1 / z_global (z summed by the AllReduce), broadcast [128, 1]
        za = small.tile([1, 2], fp32, tag="za")
        nc.vector.tensor_tensor(za[:], h1ap[0:1, HALF:HALF + 2],
                                exa[0:1, HALF:HALF + 2], op=Alu.add)
        zs = small.tile([1, 1], fp32, tag="zs")
        nc.vector.reduce_sum(zs[:], za[:], axis=mybir.AxisListType.X)
        rz = small.tile([1, 1], fp32, tag="rz")
        nc.vector.reciprocal(rz[:], zs[:])
        rz_ps = ps.tile([128, 1], fp32, tag="ps", name="rzp")
        nc.tensor.matmul(rz_ps[:], ones_row[:], rz[:], start=True, stop=True)
        rz_bc = small.tile([128, 1], fp32, tag="rz_bc")
        nc.vector.tensor_copy(rz_bc[:], rz_ps[:])

        # evict a stage's psum accs to fp32 SBUF, transpose to bf16 own tile
        def evict(accs, name, tag, scale=None, add=None, width=HALF):
            oT = med.tile([D, HALF], fp32, tag="oT32", name=f"{name}_oT")
            for blk in range(HALF // F):
                sl = slice(blk * F, (blk + 1) * F)
                if add is None:
                    nc.vector.tensor_copy(oT[:, sl], accs[blk][:])
                else:
                    nc.vector.tensor_tensor(oT[:, sl], accs[blk][:],
                                            add[:, sl], op=Alu.add)
            ex = med.tile([128, width], bf16, tag=tag, name=f"{name}_ex")
            for t in range(HT):
                transpose_cols(oT[:], t, ex[:, t * 128:(t + 1) * 128],
                               scale=scale)
            return ex

        # ------- stage 6: h1b own n-half = (H @ h1a)/z -------
        accs = stream_stage(htc_d, fp8, CH8, split_stat(exa, h1ap), "h1b")
        exb = evict(accs, "h1b", "exB", scale=rz_bc[:])
        h1bp = exchange(exb, HALF, "ex2", "B")

        # ------- stage 7: h1c own n-half = Dv @ h1b -------
        accs = stream_stage(dvT_d, bf16, CH16, split_stat(exb, h1bp), "h1c")
        exc = evict(accs, "h1c", "exA")
        h1cp = exchange(exc, HALF, "ex3", "A")

        # ------- stage 8: h1d own e-half = Ht @ h1c -------
        accs = stream_stage(hcol_d, fp8, CH8, split_stat(exc, h1cp), "h1d")
        exd = evict(accs, "h1d", "exB")
        h1dp = exchange(exd, HALF, "ex4", "B")

        # ------- stage 9+10: h own e-half = De @ h1d + eps*hx -------
        accs = stream_stage(deT_d, bf16, CH16, split_stat(exd, h1dp), "h1e")
        exe = evict(accs, "h1e", "exA", add=ehxT)
        hp = exchange(exe, HALF, "ex5", "A")

        # ------- stage 11: out own n-half = H @ h -------
        accs = stream_stage(htc_d, fp8, CH8, split_stat(exe, hp), "out")

        # ------- stage 12: epilogue: bn(leaky_relu(out)), blockwise -------
        outT = med.tile([D, HALF], fp32, tag="oT32", name="outT")
        for blk in range(HALF // F):
            sl = slice(blk * F, (blk + 1) * F)
            nc.scalar.activation(outT[:, sl], accs[blk][:], Act.Lrelu,
                                 alpha=0.01)
            nc.vector.tensor_scalar(outT[:, sl], outT[:, sl], s_bn[:],
                                    t_bn[:], op0=Alu.mult, op1=Alu.add)
            nc.sync.dma_start(out=y_d.ap()[:, sl], in_=outT[:, sl])

    nc.finalize()
    return nc


def _get_nc():
    if "nc" not in _CACHE:
        _CACHE["nc"] = _build()
    return _CACHE["nc"]


def _shard(inputs):
    import ml_dtypes
    bf16 = ml_dtypes.bfloat16
    fp8 = ml_dtypes.float8_e4m3

    H = np.asarray(inputs["incident_mat"], dtype=np.float32)
    Dv = np.asarray(inputs["degree_v"], dtype=np.float32)
    De = np.asarray(inputs["degree_e"], dtype=np.float32)
    x = np.asarray(inputs["x"], dtype=np.float32)
    em = np.asarray(inputs["e_masks"])
    w = np.asarray(inputs["mlp_W"], dtype=np.float32)
    b = np.asarray(inputs["mlp_b"], dtype=np.float32)
    th = np.asarray(inputs["theta_att"], dtype=np.float32).reshape(D)
    eps = np.full((D, 1), float(np.asarray(inputs["eps"]).reshape(-1)[0]),
                  dtype=np.float32)

    def col(v):
        return np.ascontiguousarray(
            np.asarray(v, dtype=np.float32).reshape(D, 1))

    bng, bnb = col(inputs["bn_gamma"]), col(inputs["bn_beta"])
    bnm, bnv = col(inputs["bn_mean"]), col(inputs["bn_var"])

    in_maps = []
    for core in range(NCORES):
        g, c = core // 2, core % 2
        lo, hi = c * HALF, (c + 1) * HALF
        # own-half-first row permutation for all streamed (contraction) dims
        perm = (np.r_[lo:hi, 0:lo] if c else np.r_[0:N])
        Hg = H[g]
        # host stage 1: x_w = x@W+b packed into vN tile form (perm rows)
        xw = (x[g] @ w + b)[perm]
        xwv = np.ascontiguousarray(
            xw.reshape(NT, 128, D).transpose(1, 0, 2).reshape(128, NT * D)
        ).astype(bf16)
        # attention scores Ht@(x@theta) computed on host, column form
        sc = (Hg[:, lo:hi].T @ (x[g] @ th)).astype(np.float32)
        scc = np.ascontiguousarray(sc.reshape(HT, 128).T)
        # mask in column-tile form [128, HT]
        maskc = np.ascontiguousarray(
            em[g, lo:hi].astype(np.float32).reshape(HT, 128).T)
        in_maps.append({
            "xwv": xwv,
            "sc": scc,
            "hcol": np.ascontiguousarray(Hg[perm][:, lo:hi]).astype(fp8),
            "htc": np.ascontiguousarray(Hg[lo:hi, :].T[perm]).astype(fp8),
            "dvT": np.ascontiguousarray(Dv[g][lo:hi, :].T[perm]).astype(bf16),
            "deT": np.ascontiguousarray(De[g][lo:hi, :].T[perm]).astype(bf16),
            "mask": maskc,
            "eps": eps,
            "bng": bng, "bnb": bnb, "bnm": bnm, "bnv": bnv,
        })
    return in_maps


def kernel(**inputs):
    from concourse.bass_utils import run_bass_kernel_spmd

    nc = _get_nc()
    in_maps = _shard(inputs)
    res = run_bass_kernel_spmd(nc, in_maps, list(range(NCORES)))
    out = np.empty((B, N, D), dtype=np.float32)
    for core in range(NCORES):
        g, c = core // 2, core % 2
        lo, hi = c * HALF, (c + 1) * HALF
        out[g, lo:hi, :] = res.results[core]["y"].T
    return out


# revision 34
# speedup vs baseline: 1.1310x; 1.1310x over previous
"""HGNN layer (hypergraph message passing) Trainium2 kernel, 8 NeuronCores.

Sharding: one graph per PAIR of cores (4 graphs x 2 cores), output-split:
within a pair each core owns HALF of every stage's output rows/columns.
Each stage streams its big-matrix shard as the MOVING matmul operand in
2MB chunks over HWDGE while the [*,128] intermediate sits stationary in
bf16 vN/vE tile form. The incidence matrix H is 0/1 so its two layouts
(hcol, htc) stream as fp8_e4m3 (exact, half the bytes); degree matrices
stream bf16. All streamed tensors are row-permuted OWN-HALF-FIRST on the
host so each stage accumulates its own 16 k-tiles while the pair exchange
for the partner half is still in flight. Exchanges are pair AllReduces of
the own-half bf16 tile; the partner half is recovered symmetrically as
(sum - own). The softmax normalizer z rides in the first exchange as a
hi/lo bf16 pair. x@W+b and x@theta are precomputed on the host.
"""

import numpy as np

B, N, E, D = 4, 4096, 4096, 128
HALF = N // 2
NCORES = 8
PAIRS = [[0, 1], [2, 3], [4, 5], [6, 7]]
BN_EPS = 1e-5
F = 512                 # moving free-dim per matmul
NT = N // 128           # 32 k-tiles over a full 4096 dim
HT = HALF // 128        # 16 tiles over a half (own output)
CH8 = 4                 # k-tiles per fp8 chunk (1MB)
CH16 = 4                # k-tiles per bf16 chunk (2MB)
ZW = 8                  # rider columns appended to exchange 1

_CACHE = {}


def _build():
    import concourse.bacc as bacc
    import concourse.mybir as mybir
    import concourse.tile as tile
    from concourse.masks import make_identity
    from contextlib import ExitStack

    fp32 = mybir.dt.float32
    bf16 = mybir.dt.bfloat16
    fp8 = mybir.dt.float8e4
    Act = mybir.ActivationFunctionType
    Alu = mybir.AluOpType

    nc = bacc.Bacc("TRN2", target_bir_lowering=False, debug=False,
                   num_devices=NCORES)

    # ---- per-core DRAM inputs (shards; see kernel() for host layout) ----
    xwv_d = nc.dram_tensor("xwv", [128, NT * 128], bf16, kind="ExternalInput")
    attn_d = nc.dram_tensor("attn", [128, HT], fp32, kind="ExternalInput")
    hcol_d = nc.dram_tensor("hcol", [N, HALF], fp8, kind="ExternalInput")
    htc_d = nc.dram_tensor("htc", [E, HALF], fp8, kind="ExternalInput")
    dvT_d = nc.dram_tensor("dvT", [N, HALF], bf16, kind="ExternalInput")
    deT_d = nc.dram_tensor("deT", [E, HALF], bf16, kind="ExternalInput")
    eps_d = nc.dram_tensor("eps", [D, 1], fp32, kind="ExternalInput")
    bng_d = nc.dram_tensor("bng", [D, 1], fp32, kind="ExternalInput")
    bnb_d = nc.dram_tensor("bnb", [D, 1], fp32, kind="ExternalInput")
    bnm_d = nc.dram_tensor("bnm", [D, 1], fp32, kind="ExternalInput")
    bnv_d = nc.dram_tensor("bnv", [D, 1], fp32, kind="ExternalInput")
    y_d = nc.dram_tensor("y", [D, HALF], fp32, kind="ExternalOutput")

    with tile.TileContext(nc) as tc, ExitStack() as ctx:
        const = ctx.enter_context(tc.tile_pool(name="const", bufs=1))
        stream = ctx.enter_context(tc.tile_pool(name="stream", bufs=8))
        med = ctx.enter_context(tc.tile_pool(name="med", bufs=1))
        small = ctx.enter_context(tc.tile_pool(name="small", bufs=1))
        ps = ctx.enter_context(tc.tile_pool(name="ps", bufs=8, space="PSUM"))
        dram = ctx.enter_context(tc.tile_pool(name="dram", bufs=1, space="DRAM"))

        ident = const.tile([128, 128], fp32)
        make_identity(nc, ident)

        def load_param(dt_):
            t = const.tile([D, 1], fp32, tag=dt_.name + "_p")
            nc.sync.dma_start(out=t[:], in_=dt_.ap())
            return t

        eps_t = load_param(eps_d)
        bng_t = load_param(bng_d)
        bnb_t = load_param(bnb_d)
        bnm_t = load_param(bnm_d)
        bnv_t = load_param(bnv_d)

        # bn scale s = gamma * rsqrt(var + eps_bn); shift t = beta - mean*s
        s_bn = small.tile([D, 1], fp32, tag="s_bn")
        nc.vector.tensor_scalar_add(s_bn[:], bnv_t[:], BN_EPS)
        nc.scalar.activation(s_bn[:], s_bn[:], Act.Sqrt)
        nc.vector.reciprocal(s_bn[:], s_bn[:])
        nc.vector.tensor_mul(s_bn[:], s_bn[:], bng_t[:])
        t_bn = small.tile([D, 1], fp32, tag="t_bn")
        nc.vector.tensor_mul(t_bn[:], bnm_t[:], s_bn[:])
        nc.vector.tensor_tensor(t_bn[:], bnb_t[:], t_bn[:], op=Alu.subtract)

        def transpose_cols(src, j, out_ap, scale=None):
            """PE-transpose src[:, 128j:128(j+1)] -> out_ap (optionally
            scaled per-partition by `scale` [128,1]) via psum."""
            pt = ps.tile([128, 128], fp32, tag="ps", name="pt")
            nc.tensor.transpose(pt[:], src[:, j * 128:(j + 1) * 128], ident[:])
            if scale is None:
                nc.vector.tensor_copy(out_ap, pt[:])
            else:
                nc.vector.tensor_scalar_mul(out_ap, pt[:], scale)

        # ------- stage 1 (host-precomputed): load x_wv vN tiles + xth hi/lo
        x_wv = med.tile([D, N], bf16, tag="x_wv")
        nc.sync.dma_start(out=x_wv[:], in_=xwv_d.ap())
        attnv = const.tile([128, HT], fp32)
        nc.sync.dma_start(out=attnv[:], in_=attn_d.ap())

        # ------- generic streamed stage: acc[d, own] += stat(j).T @ M[j] ---
        def stream_stage(dram_t, dt, ch, stat_fn, name, extra=None):
            accs = [ps.tile([128, F], fp32, tag="ps", name=f"{name}_a{i}")
                    for i in range(HALF // F)]
            for c in range(NT // ch):
                chk = stream.tile([128, ch * HALF], dt, tag="stream",
                                  name="chk")
                nc.sync.dma_start(
                    out=chk[:].rearrange("p (g n) -> p g n", g=ch),
                    in_=dram_t.ap()[c * ch * 128:(c + 1) * ch * 128, :]
                        .rearrange("(g p) n -> p g n", g=ch))
                for jj in range(ch):
                    j = c * ch + jj
                    st, sp = (j == 0), (j == NT - 1)
                    for blk in range(HALF // F):
                        sl = slice(jj * HALF + blk * F,
                                   jj * HALF + (blk + 1) * F)
                        nc.tensor.matmul(accs[blk][:], stat_fn(j), chk[:, sl],
                                         start=st, stop=sp)
                    if extra is not None:
                        extra(j, chk, jj)
            return accs

        # ------- exchange: pair-AllReduce of own-half bf16 tile ------------
        # Returns (own_ap_fn, partner tile); partner = sum - own.
        def exchange(ex, width, name, ab):
            ci = dram.tile([128, width], bf16, tag=f"{name}_i",
                           name=f"{name}_i")
            co = dram.tile([128, width], bf16, tag=f"{name}_o",
                           name=f"{name}_o")
            nc.sync.dma_start(out=ci[:], in_=ex[:])
            nc.gpsimd.collective_compute(
                "AllReduce", Alu.add, replica_groups=PAIRS,
                ins=[ci.opt()], outs=[co.opt()])
            summ = med.tile([128, width], bf16, tag=f"sum{ab}",
                            name=f"{name}_s")
            part = med.tile([128, width], bf16, tag=f"pr{ab}",
                            name=f"{name}_pr")
            # readback + subtract in halves: the first partner tiles
            # unblock without waiting for the full 0.5MB readback
            hw = width // 2
            for h2 in range(2):
                sl = slice(h2 * hw, width if h2 else hw)
                nc.sync.dma_start(out=summ[:, sl], in_=co[:, sl])
                nc.vector.tensor_tensor(part[:, sl], summ[:, sl],
                                        ex[:, sl], op=Alu.subtract)
            return part

        def split_stat(ex, part):
            def fn(j):
                src = ex if j < HT else part
                t = j % HT
                return src[:, t * 128:(t + 1) * 128]
            return fn

        # ------- stage 2: hxT[d, e'] = (Ht@x_w).T own e-half --------------
        hx_ps = stream_stage(
            hcol_d, fp8, CH8,
            lambda j: x_wv[:, j * 128:(j + 1) * 128], "hx")
        hxT = med.tile([D, HALF], fp32, tag="hxT")
        for blk in range(HALF // F):
            sl = slice(blk * F, (blk + 1) * F)
            nc.vector.tensor_copy(hxT[:, sl], hx_ps[blk][:])

        # eps-scaled hxT for stage 10
        ehxT = med.tile([D, HALF], fp32, tag="ehxT")
        nc.vector.tensor_scalar_mul(ehxT[:], hxT[:], eps_t[:])

        # h1a own vE tiles (bf16); attnv is host-normalized (attn/z), so
        # no z rider and no 1/z rescale anywhere downstream
        exa = med.tile([128, HALF], bf16, tag="exA")
        for t in range(HT):
            transpose_cols(hxT[:], t, exa[:, t * 128:(t + 1) * 128],
                           scale=attnv[:, t:t + 1])

        h1ap = exchange(exa, HALF, "ex1", "A")

        # evict a stage's psum accs to fp32 SBUF, transpose to bf16 own tile
        def evict(accs, name, tag, scale=None, add=None, width=HALF):
            oT = med.tile([D, HALF], fp32, tag="oT32", name=f"{name}_oT")
            for blk in range(HALF // F):
                sl = slice(blk * F, (blk + 1) * F)
                if add is None:
                    nc.vector.tensor_copy(oT[:, sl], accs[blk][:])
                else:
                    nc.vector.tensor_tensor(oT[:, sl], accs[blk][:],
                                            add[:, sl], op=Alu.add)
            ex = med.tile([128, width], bf16, tag=tag, name=f"{name}_ex")
            for t in range(HT):
                transpose_cols(oT[:], t, ex[:, t * 128:(t + 1) * 128],
                               scale=scale)
            return ex

        # ------- stage 6: h1b own n-half = (H @ h1a)/z -------
        accs = stream_stage(htc_d, fp8, CH8, split_stat(exa, h1ap), "h1b")
        exb = evict(accs, "h1b", "exB")
        h1bp = exchange(exb, HALF, "ex2", "B")

        # ------- stage 7: h1c own n-half = Dv @ h1b -------
        accs = stream_stage(dvT_d, bf16, CH16, split_stat(exb, h1bp), "h1c")
        exc = evict(accs, "h1c", "exA")
        h1cp = exchange(exc, HALF, "ex3", "A")

        # ------- stage 8: h1d own e-half = Ht @ h1c -------
        accs = stream_stage(hcol_d, fp8, CH8, split_stat(exc, h1cp), "h1d")
        exd = evict(accs, "h1d", "exB")
        h1dp = exchange(exd, HALF, "ex4", "B")

        # ------- stage 9+10: h own e-half = De @ h1d + eps*hx -------
        accs = stream_stage(deT_d, bf16, CH16, split_stat(exd, h1dp), "h1e")
        exe = evict(accs, "h1e", "exA", add=ehxT)
        hp = exchange(exe, HALF, "ex5", "A")

        # ------- stage 11: out own n-half = H @ h -------
        accs = stream_stage(htc_d, fp8, CH8, split_stat(exe, hp), "out")

        # ------- stage 12: epilogue: bn(leaky_relu(out)), blockwise -------
        outT = med.tile([D, HALF], fp32, tag="oT32", name="outT")
        for blk in range(HALF // F):
            sl = slice(blk * F, (blk + 1) * F)
            nc.scalar.activation(outT[:, sl], accs[blk][:], Act.Lrelu,
                                 alpha=0.01)
            nc.vector.tensor_scalar(outT[:, sl], outT[:, sl], s_bn[:],
                                    t_bn[:], op0=Alu.mult, op1=Alu.add)
            nc.sync.dma_start(out=y_d.ap()[:, sl], in_=outT[:, sl])

    nc.finalize()
    return nc


def _get_nc():
    if "nc" not in _CACHE:
        _CACHE["nc"] = _build()
    return _CACHE["nc"]


def _shard(inputs):
    import ml_dtypes
    bf16 = ml_dtypes.bfloat16
    fp8 = ml_dtypes.float8_e4m3

    H = np.asarray(inputs["incident_mat"], dtype=np.float32)
    Dv = np.asarray(inputs["degree_v"], dtype=np.float32)
    De = np.asarray(inputs["degree_e"], dtype=np.float32)
    x = np.asarray(inputs["x"], dtype=np.float32)
    em = np.asarray(inputs["e_masks"])
    w = np.asarray(inputs["mlp_W"], dtype=np.float32)
    b = np.asarray(inputs["mlp_b"], dtype=np.float32)
    th = np.asarray(inputs["theta_att"], dtype=np.float32).reshape(D)
    eps = np.full((D, 1), float(np.asarray(inputs["eps"]).reshape(-1)[0]),
                  dtype=np.float32)

    def col(v):
        return np.ascontiguousarray(
            np.asarray(v, dtype=np.float32).reshape(D, 1))

    bng, bnb = col(inputs["bn_gamma"]), col(inputs["bn_beta"])
    bnm, bnv = col(inputs["bn_mean"]), col(inputs["bn_var"])

    in_maps = []
    for core in range(NCORES):
        g, c = core // 2, core % 2
        lo, hi = c * HALF, (c + 1) * HALF
        # own-half-first row permutation for all streamed (contraction) dims
        perm = (np.r_[lo:hi, 0:lo] if c else np.r_[0:N])
        Hg = H[g]
        # host stage 1: x_w = x@W+b packed into vN tile form (perm rows)
        xw = (x[g] @ w + b)[perm]
        xwv = np.ascontiguousarray(
            xw.reshape(NT, 128, D).transpose(1, 0, 2).reshape(128, NT * D)
        ).astype(bf16)
        # full softmax attention computed on host (stable), column form
        sfull = (Hg.T @ (x[g] @ th)).astype(np.float64)
        mfull = em[g] != 0
        a = np.where(mfull, np.exp(sfull - sfull[mfull].max()), 0.0)
        attn = (a / a.sum()).astype(np.float32)[lo:hi]
        attc = np.ascontiguousarray(attn.reshape(HT, 128).T)
        in_maps.append({
            "xwv": xwv,
            "attn": attc,
            "hcol": np.ascontiguousarray(Hg[perm][:, lo:hi]).astype(fp8),
            "htc": np.ascontiguousarray(Hg[lo:hi, :].T[perm]).astype(fp8),
            "dvT": np.ascontiguousarray(Dv[g][lo:hi, :].T[perm]).astype(bf16),
            "deT": np.ascontiguousarray(De[g][lo:hi, :].T[perm]).astype(bf16),
            "eps": eps,
            "bng": bng, "bnb": bnb, "bnm": bnm, "bnv": bnv,
        })
    return in_maps


def kernel(**inputs):
    from concourse.bass_utils import run_bass_kernel_spmd

    nc = _get_nc()
    in_maps = _shard(inputs)
    res = run_bass_kernel_spmd(nc, in_maps, list(range(NCORES)))
    out = np.empty((B, N, D), dtype=np.float32)
    for core in range(NCORES):
        g, c = core // 2, core % 2
        lo, hi = c * HALF, (c + 1) * HALF
        out[g, lo:hi, :] = res.results[core]["y"].T
    return out


# revision 36
# speedup vs baseline: 1.1409x; 1.0087x over previous
"""HGNN layer (hypergraph message passing) Trainium2 kernel, 8 NeuronCores.

Sharding: one graph per PAIR of cores (4 graphs x 2 cores), output-split:
within a pair each core owns HALF of every stage's output rows/columns.
Each stage streams its big-matrix shard as the MOVING matmul operand in
2MB chunks over HWDGE while the [*,128] intermediate sits stationary in
bf16 vN/vE tile form. The incidence matrix H is 0/1 so its two layouts
(hcol, htc) stream as fp8_e4m3 (exact, half the bytes); degree matrices
stream bf16. All streamed tensors are row-permuted OWN-HALF-FIRST on the
host so each stage accumulates its own 16 k-tiles while the pair exchange
for the partner half is still in flight. Exchanges are pair AllReduces of
the own-half bf16 tile; the partner half is recovered symmetrically as
(sum - own), with the readback+subtract split in halves so the first
partner tiles unblock early. x@W+b and the full normalized softmax
attention (exact fp32) are precomputed on the host.
"""

import numpy as np

B, N, E, D = 4, 4096, 4096, 128
HALF = N // 2
NCORES = 8
PAIRS = [[0, 1], [2, 3], [4, 5], [6, 7]]
BN_EPS = 1e-5
F = 512                 # moving free-dim per matmul
NT = N // 128           # 32 k-tiles over a full 4096 dim
HT = HALF // 128        # 16 tiles over a half (own output)
CH8 = 2                 # k-tiles per fp8 chunk (0.5MB)
CH16 = 4                # k-tiles per bf16 chunk (2MB)

_CACHE = {}


def _build():
    import concourse.bacc as bacc
    import concourse.mybir as mybir
    import concourse.tile as tile
    from concourse.masks import make_identity
    from contextlib import ExitStack

    fp32 = mybir.dt.float32
    bf16 = mybir.dt.bfloat16
    fp8 = mybir.dt.float8e4
    Act = mybir.ActivationFunctionType
    Alu = mybir.AluOpType

    nc = bacc.Bacc("TRN2", target_bir_lowering=False, debug=False,
                   num_devices=NCORES)

    # ---- per-core DRAM inputs (shards; see kernel() for host layout) ----
    xwv_d = nc.dram_tensor("xwv", [128, NT * 128], bf16, kind="ExternalInput")
    attn_d = nc.dram_tensor("attn", [128, HT], fp32, kind="ExternalInput")
    hcol_d = nc.dram_tensor("hcol", [N, HALF], fp8, kind="ExternalInput")
    htc_d = nc.dram_tensor("htc", [E, HALF], fp8, kind="ExternalInput")
    dvT_d = nc.dram_tensor("dvT", [N, HALF], bf16, kind="ExternalInput")
    deT_d = nc.dram_tensor("deT", [E, HALF], bf16, kind="ExternalInput")
    eps_d = nc.dram_tensor("eps", [D, 1], fp32, kind="ExternalInput")
    bng_d = nc.dram_tensor("bng", [D, 1], fp32, kind="ExternalInput")
    bnb_d = nc.dram_tensor("bnb", [D, 1], fp32, kind="ExternalInput")
    bnm_d = nc.dram_tensor("bnm", [D, 1], fp32, kind="ExternalInput")
    bnv_d = nc.dram_tensor("bnv", [D, 1], fp32, kind="ExternalInput")
    y_d = nc.dram_tensor("y", [D, HALF], fp32, kind="ExternalOutput")

    with tile.TileContext(nc) as tc, ExitStack() as ctx:
        const = ctx.enter_context(tc.tile_pool(name="const", bufs=1))
        stream = ctx.enter_context(tc.tile_pool(name="stream", bufs=9))
        med = ctx.enter_context(tc.tile_pool(name="med", bufs=1))
        small = ctx.enter_context(tc.tile_pool(name="small", bufs=1))
        ps = ctx.enter_context(tc.tile_pool(name="ps", bufs=8, space="PSUM"))
        dram = ctx.enter_context(tc.tile_pool(name="dram", bufs=1, space="DRAM"))

        ident = const.tile([128, 128], fp32)
        make_identity(nc, ident)

        def load_param(dt_):
            t = const.tile([D, 1], fp32, tag=dt_.name + "_p")
            nc.sync.dma_start(out=t[:], in_=dt_.ap())
            return t

        eps_t = load_param(eps_d)
        bng_t = load_param(bng_d)
        bnb_t = load_param(bnb_d)
        bnm_t = load_param(bnm_d)
        bnv_t = load_param(bnv_d)

        # bn scale s = gamma * rsqrt(var + eps_bn); shift t = beta - mean*s
        s_bn = small.tile([D, 1], fp32, tag="s_bn")
        nc.vector.tensor_scalar_add(s_bn[:], bnv_t[:], BN_EPS)
        nc.scalar.activation(s_bn[:], s_bn[:], Act.Sqrt)
        nc.vector.reciprocal(s_bn[:], s_bn[:])
        nc.vector.tensor_mul(s_bn[:], s_bn[:], bng_t[:])
        t_bn = small.tile([D, 1], fp32, tag="t_bn")
        nc.vector.tensor_mul(t_bn[:], bnm_t[:], s_bn[:])
        nc.vector.tensor_tensor(t_bn[:], bnb_t[:], t_bn[:], op=Alu.subtract)

        def transpose_cols(src, j, out_ap, scale=None):
            """PE-transpose src[:, 128j:128(j+1)] -> out_ap (optionally
            scaled per-partition by `scale` [128,1]) via psum."""
            pt = ps.tile([128, 128], fp32, tag="ps", name="pt")
            nc.tensor.transpose(pt[:], src[:, j * 128:(j + 1) * 128], ident[:])
            if scale is None:
                nc.vector.tensor_copy(out_ap, pt[:])
            else:
                nc.vector.tensor_scalar_mul(out_ap, pt[:], scale)

        # ------- stage 1 (host-precomputed): load x_wv vN tiles + xth hi/lo
        x_wv = med.tile([D, N], bf16, tag="x_wv")
        nc.sync.dma_start(out=x_wv[:], in_=xwv_d.ap())
        attnv = const.tile([128, HT], fp32)
        nc.sync.dma_start(out=attnv[:], in_=attn_d.ap())

        # ------- generic streamed stage: acc[d, own] += stat(j).T @ M[j] ---
        def stream_stage(dram_t, dt, ch, stat_fn, name, extra=None):
            accs = [ps.tile([128, F], fp32, tag="ps", name=f"{name}_a{i}")
                    for i in range(HALF // F)]
            for c in range(NT // ch):
                chk = stream.tile([128, ch * HALF], dt, tag="stream",
                                  name="chk")
                nc.sync.dma_start(
                    out=chk[:].rearrange("p (g n) -> p g n", g=ch),
                    in_=dram_t.ap()[c * ch * 128:(c + 1) * ch * 128, :]
                        .rearrange("(g p) n -> p g n", g=ch))
                for jj in range(ch):
                    j = c * ch + jj
                    st, sp = (j == 0), (j == NT - 1)
                    for blk in range(HALF // F):
                        sl = slice(jj * HALF + blk * F,
                                   jj * HALF + (blk + 1) * F)
                        nc.tensor.matmul(accs[blk][:], stat_fn(j), chk[:, sl],
                                         start=st, stop=sp)
                    if extra is not None:
                        extra(j, chk, jj)
            return accs

        # ------- exchange: pair-AllReduce of own-half bf16 tile ------------
        # Returns (own_ap_fn, partner tile); partner = sum - own.
        def exchange(ex, width, name, ab):
            ci = dram.tile([128, width], bf16, tag=f"{name}_i",
                           name=f"{name}_i")
            co = dram.tile([128, width], bf16, tag=f"{name}_o",
                           name=f"{name}_o")
            nc.sync.dma_start(out=ci[:], in_=ex[:])
            nc.gpsimd.collective_compute(
                "AllReduce", Alu.add, replica_groups=PAIRS,
                ins=[ci.opt()], outs=[co.opt()])
            summ = med.tile([128, width], bf16, tag=f"sum{ab}",
                            name=f"{name}_s")
            part = med.tile([128, width], bf16, tag=f"pr{ab}",
                            name=f"{name}_pr")
            # readback + subtract in halves: the first partner tiles
            # unblock without waiting for the full 0.5MB readback
            hw = width // 2
            for h2 in range(2):
                sl = slice(h2 * hw, width if h2 else hw)
                nc.sync.dma_start(out=summ[:, sl], in_=co[:, sl])
                nc.vector.tensor_tensor(part[:, sl], summ[:, sl],
                                        ex[:, sl], op=Alu.subtract)
            return part

        def split_stat(ex, part):
            def fn(j):
                src = ex if j < HT else part
                t = j % HT
                return src[:, t * 128:(t + 1) * 128]
            return fn

        # ------- stage 2: hxT[d, e'] = (Ht@x_w).T own e-half --------------
        hx_ps = stream_stage(
            hcol_d, fp8, CH8,
            lambda j: x_wv[:, j * 128:(j + 1) * 128], "hx")
        hxT = med.tile([D, HALF], fp32, tag="hxT")
        for blk in range(HALF // F):
            sl = slice(blk * F, (blk + 1) * F)
            nc.vector.tensor_copy(hxT[:, sl], hx_ps[blk][:])

        # eps-scaled hxT for stage 10
        ehxT = med.tile([D, HALF], fp32, tag="ehxT")
        nc.vector.tensor_scalar_mul(ehxT[:], hxT[:], eps_t[:])

        # h1a own vE tiles (bf16); attnv is host-normalized (attn/z), so
        # no z rider and no 1/z rescale anywhere downstream
        exa = med.tile([128, HALF], bf16, tag="exA")
        for t in range(HT):
            transpose_cols(hxT[:], t, exa[:, t * 128:(t + 1) * 128],
                           scale=attnv[:, t:t + 1])

        h1ap = exchange(exa, HALF, "ex1", "A")

        # evict a stage's psum accs to fp32 SBUF, transpose to bf16 own tile
        def evict(accs, name, tag, scale=None, add=None, width=HALF):
            oT = med.tile([D, HALF], fp32, tag="oT32", name=f"{name}_oT")
            for blk in range(HALF // F):
                sl = slice(blk * F, (blk + 1) * F)
                if add is None:
                    nc.vector.tensor_copy(oT[:, sl], accs[blk][:])
                else:
                    nc.vector.tensor_tensor(oT[:, sl], accs[blk][:],
                                            add[:, sl], op=Alu.add)
            ex = med.tile([128, width], bf16, tag=tag, name=f"{name}_ex")
            for t in range(HT):
                transpose_cols(oT[:], t, ex[:, t * 128:(t + 1) * 128],
                               scale=scale)
            return ex

        # ------- stage 6: h1b own n-half = (H @ h1a)/z -------
        accs = stream_stage(htc_d, fp8, CH8, split_stat(exa, h1ap), "h1b")
        exb = evict(accs, "h1b", "exB")
        h1bp = exchange(exb, HALF, "ex2", "B")

        # ------- stage 7: h1c own n-half = Dv @ h1b -------
        accs = stream_stage(dvT_d, bf16, CH16, split_stat(exb, h1bp), "h1c")
        exc = evict(accs, "h1c", "exA")
        h1cp = exchange(exc, HALF, "ex3", "A")

        # ------- stage 8: h1d own e-half = Ht @ h1c -------
        accs = stream_stage(hcol_d, fp8, CH8, split_stat(exc, h1cp), "h1d")
        exd = evict(accs, "h1d", "exB")
        h1dp = exchange(exd, HALF, "ex4", "B")

        # ------- stage 9+10: h own e-half = De @ h1d + eps*hx -------
        accs = stream_stage(deT_d, bf16, CH16, split_stat(exd, h1dp), "h1e")
        exe = evict(accs, "h1e", "exA", add=ehxT)
        hp = exchange(exe, HALF, "ex5", "A")

        # ------- stage 11: out own n-half = H @ h -------
        accs = stream_stage(htc_d, fp8, CH8, split_stat(exe, hp), "out")

        # ------- stage 12: epilogue: bn(leaky_relu(out)), blockwise -------
        outT = med.tile([D, HALF], fp32, tag="oT32", name="outT")
        for blk in range(HALF // F):
            sl = slice(blk * F, (blk + 1) * F)
            nc.scalar.activation(outT[:, sl], accs[blk][:], Act.Lrelu,
                                 alpha=0.01)
            nc.vector.tensor_scalar(outT[:, sl], outT[:, sl], s_bn[:],
                                    t_bn[:], op0=Alu.mult, op1=Alu.add)
            nc.sync.dma_start(out=y_d.ap()[:, sl], in_=outT[:, sl])

    nc.finalize()
    return nc


def _get_nc():
    if "nc" not in _CACHE:
        _CACHE["nc"] = _build()
    return _CACHE["nc"]


def _shard(inputs):
    import ml_dtypes
    bf16 = ml_dtypes.bfloat16
    fp8 = ml_dtypes.float8_e4m3

    H = np.asarray(inputs["incident_mat"], dtype=np.float32)
    Dv = np.asarray(inputs["degree_v"], dtype=np.float32)
    De = np.asarray(inputs["degree_e"], dtype=np.float32)
    x = np.asarray(inputs["x"], dtype=np.float32)
    em = np.asarray(inputs["e_masks"])
    w = np.asarray(inputs["mlp_W"], dtype=np.float32)
    b = np.asarray(inputs["mlp_b"], dtype=np.float32)
    th = np.asarray(inputs["theta_att"], dtype=np.float32).reshape(D)
    eps = np.full((D, 1), float(np.asarray(inputs["eps"]).reshape(-1)[0]),
                  dtype=np.float32)

    def col(v):
        return np.ascontiguousarray(
            np.asarray(v, dtype=np.float32).reshape(D, 1))

    bng, bnb = col(inputs["bn_gamma"]), col(inputs["bn_beta"])
    bnm, bnv = col(inputs["bn_mean"]), col(inputs["bn_var"])

    in_maps = []
    for core in range(NCORES):
        g, c = core // 2, core % 2
        lo, hi = c * HALF, (c + 1) * HALF
        # own-half-first row permutation for all streamed (contraction) dims
        perm = (np.r_[lo:hi, 0:lo] if c else np.r_[0:N])
        Hg = H[g]
        # host stage 1: x_w = x@W+b packed into vN tile form (perm rows)
        xw = (x[g] @ w + b)[perm]
        xwv = np.ascontiguousarray(
            xw.reshape(NT, 128, D).transpose(1, 0, 2).reshape(128, NT * D)
        ).astype(bf16)
        # full softmax attention computed on host (stable), column form
        sfull = (Hg.T @ (x[g] @ th)).astype(np.float64)
        mfull = em[g] != 0
        a = np.where(mfull, np.exp(sfull - sfull[mfull].max()), 0.0)
        attn = (a / a.sum()).astype(np.float32)[lo:hi]
        attc = np.ascontiguousarray(attn.reshape(HT, 128).T)
        in_maps.append({
            "xwv": xwv,
            "attn": attc,
            "hcol": np.ascontiguousarray(Hg[perm][:, lo:hi]).astype(fp8),
            "htc": np.ascontiguousarray(Hg[lo:hi, :].T[perm]).astype(fp8),
            "dvT": np.ascontiguousarray(Dv[g][lo:hi, :].T[perm]).astype(bf16),
            "deT": np.ascontiguousarray(De[g][lo:hi, :].T[perm]).astype(bf16),
            "eps": eps,
            "bng": bng, "bnb": bnb, "bnm": bnm, "bnv": bnv,
        })
    return in_maps


def kernel(**inputs):
    from concourse.bass_utils import run_bass_kernel_spmd

    nc = _get_nc()
    in_maps = _shard(inputs)
    res = run_bass_kernel_spmd(nc, in_maps, list(range(NCORES)))
    out = np.empty((B, N, D), dtype=np.float32)
    for core in range(NCORES):
        g, c = core // 2, core % 2
        lo, hi = c * HALF, (c + 1) * HALF
        out[g, lo:hi, :] = res.results[core]["y"].T
    return out
